# revision 1
# baseline (speedup 1.0000x reference)
"""Otsu-threshold binary region proposal kernel for Trainium2 (8 NeuronCores).

Algorithm (per image of 224*224 pixels, 512 images total, data-parallel over
8 cores / 64 images per core):

  reference:  cam = floor(x*255); per-image 256-bin histogram; Otsu threshold
              via argmax of inter-class variance restricted to [vmin, vmax);
              roi = (cam > th), 0 for degenerate images.

Device pass A (histogram):
  A 256-bin histogram is too expensive elementwise, so we use a thermometer
  decomposition: with hi = cam >> 4, lo = cam & 15,
      R[tau, sigma] = sum_p colA_tau(p) * colB_sigma(p)
  where colA_tau ~ [hi >= tau] and colB_sigma ~ [lo >= sigma] are built on
  DVE / ACT / GPSIMD (16+16 cut columns, bf16), and the 16x16 pair-count
  matrix is accumulated on the TensorEngine (one [128,16]x[128,16] matmul per
  128-pixel chunk into PSUM).  ACT's columns are +-1 coded (Sign), DVE /
  GPSIMD's are 0/1 coded (is_le); the host decodes mixed codings exactly via
  the marginal row/col (tau=0 / sigma=0 are always-true cuts).
  All counts are exact small integers in fp32/bf16.

Host (exact float32, mirrors jax reference op-for-op):
  W -> 2D difference -> 256-bin histogram -> cumsums -> inter-class variance
  -> argmax -> threshold; then fold "cam > th" into a single fp32 cut on raw
  x (monotonicity of x -> floor(fl(255x)) makes this exact).

Device pass B (mask): mask = (x >= dth_image) as uint8, streamed at memory
speed.  Host casts to int32.

floor() trick (no floor ALU op): negcam = fmod(255x, 1) - 255x = -floor(255x)
exactly in fp32; neglo = fmod(negcam, 16) = -(cam mod 16).  Comparisons then
use is_le against negated cuts; integers up to 255 are exact in bf16.
"""

import math
import os
import sys

import numpy as np

sys.path.insert(0, "/opt/trn_rl_repo")

import concourse.bacc as bacc
import concourse.bass as bass  # noqa: F401
import concourse.mybir as mybir
from concourse.bass_utils import run_bass_kernel_spmd
from concourse.tile import TileContext

# ---------------------------------------------------------------------------
# Problem geometry (hardcoded per spec)
B, N, H, W_IMG = 64, 8, 224, 224
PIX = H * W_IMG              # 50176
PARTS = 128
CPI = PIX // PARTS           # 392 chunks (columns) per image
N_CORES = 8
IMGS_PER_CORE = (B // N_CORES) * N      # 64
NBINS = 256

# Tunables
GROUP = 3          # images per thermo group (instruction batching)
PSUM_G = 8         # unused (kept for dev_sim compat)
# Cut assignment: plane A rows tau=0..15 (cut on cam at 16*tau), plane B rows
# sigma=0..15 (cut on lo at sigma).  B lives cut-major (contiguous writes ->
# DVE 4x) and feeds the matmul as the moving operand; A lives pack-major
# (the stationary operand needs a single-free-dim [1,128] AP) where DVE
# writes are 2x, so the pricier A rows split between ACT (Sign, +-1) and
# the otherwise-idle GPSIMD (is_ge, 0/1).
ACT_A_START = 7    # A rows [this, GP_A_START) run on ACT; rows 1..this-1 DVE
GP_A_START = 16    # A rows >= this run on GPSIMD (16 = none: Q7 strided
                   # is_ge measured 17us/op and its SBUF-port contention
                   # drops DVE 4x-mode rows to 1x — never enable)
ACT_B_START = 16   # B rows >= this run on ACT (none by default)

FP32 = mybir.dt.float32
BF16 = mybir.dt.bfloat16
I16 = mybir.dt.int16
I8 = mybir.dt.int8
U8 = mybir.dt.uint8
ALU = mybir.AluOpType
ACTF = mybir.ActivationFunctionType
MAGIC = 8388608.0  # 2^23: fp32 ulp 1.0 => fl(s + MAGIC) = MAGIC + round(s)


def _enc_pm():
    """Which rows are +-1 coded (ACT Sign)."""
    encA = np.zeros(16, dtype=bool)
    encB = np.zeros(16, dtype=bool)
    encA[ACT_A_START:GP_A_START] = True
    encB[ACT_B_START:] = True
    return encA, encB


# ---------------------------------------------------------------------------
# Pass A: histogram kernel
def build_hist_nc(nimg=IMGS_PER_CORE, cpi=CPI, group=GROUP, psum_g=PSUM_G):
    """Pass A.  Thermo tiles are laid out [128, W/8, 16, 8] so that each
    8-chunk pack is one contiguous [128, 128] block: the PE then runs one
    [128,128]x[128,128] matmul per 8 chunks (block-diagonal trick — psum row
    8*tau+c', col 8*sigma+c''; only c'==c'' blocks are meaningful and the
    host sums them).  N=16 matmuls were drain-bound at ~47ns; packed N=128
    matmuls measure ~69ns for 8x the work."""
    assert cpi % 8 == 0
    nc = bacc.Bacc("TRN2", target_bir_lowering=False, debug=False)
    # partition-major DRAM layout: x[p, i, c] -> contiguous multi-KB runs per
    # partition line (fat DMA descriptors instead of 1.5KB ones)
    x_d = nc.dram_tensor("x", [PARTS, nimg, cpi], FP32, kind="ExternalInput")
    w_d = nc.dram_tensor("w_raw", [nimg, PARTS, PARTS], FP32, kind="ExternalOutput")

    with TileContext(nc) as tc:
        with (
            tc.tile_pool(name="const", bufs=1) as cpool,
            tc.tile_pool(name="xin", bufs=2) as xpool,
            tc.tile_pool(name="prep", bufs=2) as ppool,
            tc.tile_pool(name="thermo", bufs=2) as tpool,
            tc.tile_pool(name="psum", bufs=4, space="PSUM") as qpool,
        ):
            nACT = (GP_A_START - ACT_A_START) + (16 - ACT_B_START)
            act_bias = cpool.tile([PARTS, max(nACT, 1)], FP32, tag="abias")
            ab_idx = {}
            j = 0
            for tau in range(ACT_A_START, GP_A_START):
                nc.vector.memset(act_bias[:, j:j + 1], 0.5 - 16 * tau)
                ab_idx[("A", tau)] = j
                j += 1
            for sg in range(ACT_B_START, 16):
                nc.vector.memset(act_bias[:, j:j + 1], 0.5 - sg)
                ab_idx[("B", sg)] = j
                j += 1

            n_groups = math.ceil(nimg / group)
            for g in range(n_groups):
                g0 = g * group
                g1 = min(g0 + group, nimg)
                gw = (g1 - g0) * cpi
                gw8 = gw // 8

                x_t = xpool.tile([PARTS, group * cpi], FP32, tag="x")
                ci = ppool.tile([PARTS, group * cpi], I16, tag="ci")
                lo = ppool.tile([PARTS, group * cpi], I16, tag="lo")
                A_t = tpool.tile([PARTS, group * cpi // 8, 16, 8], BF16, tag="A")
                B_t = tpool.tile([PARTS, 16, group * cpi], BF16, tag="B")

                # one batched load for the whole group (partition-major src)
                nc.sync.dma_start(
                    out=x_t[:, :gw],
                    in_=bass.AP(
                        x_d, g0 * cpi,
                        [[nimg * cpi, PARTS], [1, gw]],
                    ),
                )

                # ACT: s = fl(255x) in-place; then one-op exact floor:
                # ci = fl(s + (2^23 - 0.5)) - 2^23 = floor(s)  (s integer only
                # at s=0, where the half-even tie also lands on 0)
                nc.scalar.activation(
                    out=x_t[:, :gw], in_=x_t[:, :gw],
                    func=ACTF.Copy, bias=0.0, scale=255.0,
                )
                nc.vector.tensor_scalar(
                    out=ci[:, :gw], in0=x_t[:, :gw],
                    scalar1=MAGIC - 0.5, scalar2=-MAGIC,
                    op0=ALU.add, op1=ALU.add,
                )
                nc.vector.tensor_scalar(
                    out=lo[:, :gw], in0=ci[:, :gw],
                    scalar1=15, scalar2=None, op0=ALU.bitwise_and,
                )

                ci_v = ci[:, :gw].rearrange("p (a b) -> p a b", b=8)
                # A plane (pack-major, [1,8]-run writes = DVE 2x / ACT Sign /
                # GPSIMD is_ge)
                nc.gpsimd.memset(A_t[:, :gw8, 0, :], 1.0)  # tau=0 always true
                for tau in range(1, ACT_A_START):
                    nc.vector.tensor_scalar(
                        out=A_t[:, :gw8, tau, :], in0=ci_v,
                        scalar1=16 * tau, scalar2=None, op0=ALU.is_ge,
                    )
                for tau in range(ACT_A_START, GP_A_START):
                    nc.scalar.activation(
                        out=A_t[:, :gw8, tau, :], in_=ci_v,
                        func=ACTF.Sign,
                        bias=act_bias[:, ab_idx[("A", tau)]:ab_idx[("A", tau)] + 1],
                        scale=1.0,
                    )
                for tau in range(GP_A_START, 16):
                    nc.gpsimd.tensor_scalar(
                        out=A_t[:, :gw8, tau, :], in0=ci_v,
                        scalar1=16 * tau, scalar2=None, op0=ALU.is_ge,
                    )
                # B plane (cut-major, contiguous writes = DVE 4x)
                nc.gpsimd.memset(B_t[:, 0, :gw], 1.0)  # sigma=0 always true
                for sg in range(1, ACT_B_START):
                    nc.vector.tensor_scalar(
                        out=B_t[:, sg, :gw], in0=lo[:, :gw],
                        scalar1=sg, scalar2=None, op0=ALU.is_ge,
                    )
                for sg in range(ACT_B_START, 16):
                    nc.scalar.activation(
                        out=B_t[:, sg, :gw], in_=lo[:, :gw],
                        func=ACTF.Sign,
                        bias=act_bias[:, ab_idx[("B", sg)]:ab_idx[("B", sg)] + 1],
                        scale=1.0,
                    )

                # PE: per image, 49 packed [128,128] matmuls accumulate in PSUM
                packs_per_img = cpi // 8
                for i in range(g0, g1):
                    il = i - g0
                    psum_t = qpool.tile([PARTS, PARTS], FP32, tag="ps")
                    for k in range(packs_per_img):
                        p = il * packs_per_img + k
                        nc.tensor.matmul(
                            psum_t[:],
                            A_t[:, p, :, :].rearrange("p a b -> p (a b)"),
                            B_t[:, :, 8 * p:8 * p + 8],
                            start=(k == 0),
                            stop=(k == packs_per_img - 1),
                        )
                    w_sb = ppool.tile([PARTS, PARTS], FP32, tag="wsb")
                    if i % 2 == 0:
                        nc.scalar.copy(w_sb[:], psum_t[:])
                    else:
                        nc.vector.tensor_copy(out=w_sb[:], in_=psum_t[:])
                    nc.sync.dma_start(out=w_d.ap()[i], in_=w_sb[:])
    nc.finalize()
    return nc


# ---------------------------------------------------------------------------
# Pass B: mask kernel
def build_mask_nc(nimg=IMGS_PER_CORE, cpi=CPI, mgroup=16):
    nc = bacc.Bacc("TRN2", target_bir_lowering=False, debug=False)
    # partition-major DRAM layouts (fat contiguous DMA runs per partition)
    x_d = nc.dram_tensor("x", [PARTS, nimg, cpi], FP32, kind="ExternalInput")
    t_d = nc.dram_tensor("dth", [nimg, PARTS], FP32, kind="ExternalInput")
    m_d = nc.dram_tensor("mask", [PARTS, nimg, cpi], BF16, kind="ExternalOutput")

    with TileContext(nc) as tc:
        with (
            tc.tile_pool(name="cst", bufs=1) as cpool,
            tc.tile_pool(name="xin", bufs=4) as xpool,
            tc.tile_pool(name="mo", bufs=4) as mpool,
        ):
            # all thresholds in one transfer: sbuf[p, i] = dth[i, p]
            th_all = cpool.tile([PARTS, nimg], FP32, tag="t")
            nc.sync.dma_start(
                out=th_all[:],
                in_=bass.AP(t_d, 0, [[1, PARTS], [PARTS, nimg]]),
            )
            for g0 in range(0, nimg, mgroup):
                g1 = min(g0 + mgroup, nimg)
                gl = g1 - g0
                # flat [P, g*cpi] tiles + 2-dim APs: one fat contiguous run
                # per partition line (no descriptor splitting on inner dims)
                x_t = xpool.tile([PARTS, mgroup * cpi], FP32, tag="x")
                m_t = mpool.tile([PARTS, mgroup * cpi], BF16, tag="m")
                nc.sync.dma_start(
                    out=x_t[:, :gl * cpi],
                    in_=bass.AP(
                        x_d, g0 * cpi,
                        [[nimg * cpi, PARTS], [1, gl * cpi]],
                    ),
                )
                for i in range(g0, g1):
                    il = i - g0
                    nc.vector.tensor_scalar(
                        out=m_t[:, il * cpi:(il + 1) * cpi],
                        in0=x_t[:, il * cpi:(il + 1) * cpi],
                        scalar1=th_all[:, i:i + 1],
                        scalar2=None, op0=ALU.is_ge,
                    )
                nc.sync.dma_start(
                    out=bass.AP(
                        m_d, g0 * cpi,
                        [[nimg * cpi, PARTS], [1, gl * cpi]],
                    ),
                    in_=m_t[:, :gl * cpi],
                )
    nc.finalize()
    return nc


# ---------------------------------------------------------------------------
# Host: decode W, exact-float32 Otsu, threshold folding
def decode_hist(w_raw, nimg=IMGS_PER_CORE, npix=PIX):
    """w_raw [nimg, 128, 128] fp32 -> hist [nimg, 256] int64 (exact).

    Psum row 8*tau+c', col 8*sigma+c'': sum the c'==c'' diagonal blocks."""
    encA, encB = _enc_pm()
    P128 = np.round(np.asarray(w_raw, np.float64)).astype(np.int64)
    P128 = P128.reshape(nimg, 16, 8, 16, 8)  # [img, tau, c', sigma, c'']
    R = np.einsum("itcsc->its", P128)        # [img, tau, sigma]
    P = npix
    # marginals from always-true rows (tau=0 / sigma=0 columns are exact ones)
    sumB = np.where(encB[None, :], (R[:, 0, :] + P) // 2, R[:, 0, :])  # [img,16]
    sumA = np.where(encA[None, :], (R[:, :, 0] + P) // 2, R[:, :, 0])  # [img,16]
    eA = encA[None, :, None]
    eB = encB[None, None, :]
    sA = sumA[:, :, None]
    sB = sumB[:, None, :]
    W = np.where(
        ~eA & ~eB, R,
        np.where(
            eA & ~eB, (R + sB) // 2,
            np.where(~eA & eB, (R + sA) // 2, (R + 2 * sA + 2 * sB - P) // 4),
        ),
    )
    # sanity: the integer divisions above must be exact
    chk = np.where(
        ~eA & ~eB, 0,
        np.where(eA & ~eB, (R + sB) % 2,
                 np.where(~eA & eB, (R + sA) % 2, (R + 2 * sA + 2 * sB - P) % 4)),
    )
    assert not chk.any(), "non-integer decode: device histogram corrupted"
    Wp = np.zeros((nimg, 17, 17), np.int64)
    Wp[:, :16, :16] = W
    hist = (Wp[:, :16, :16] - Wp[:, 1:, :16] - Wp[:, :16, 1:] + Wp[:, 1:, 1:])
    hist = hist.reshape(nimg, 256)
    assert (hist >= 0).all() and (hist.sum(1) == P).all(), "bad histogram"
    return hist


def otsu_f32(hist):
    """Mirror the jax float32 reference exactly. hist [n, 256] int64 -> th int, bad mask."""
    f = hist.astype(np.float32)
    centers = np.arange(NBINS, dtype=np.float32)
    w1 = np.cumsum(f, axis=1, dtype=np.float32)
    total = w1[:, -1:]
    s1 = np.cumsum(f * centers, axis=1, dtype=np.float32)
    stot = s1[:, -1:]
    w2 = total - w1
    with np.errstate(divide="ignore", invalid="ignore"):
        m1 = s1 / w1
        m2 = (stot - s1) / w2
        d = m1 - m2
        var12 = (w1 * w2) * (d * d)
    nz = hist > 0
    t = np.arange(NBINS)
    vmin = np.argmax(nz, axis=1)
    vmax = NBINS - 1 - np.argmax(nz[:, ::-1], axis=1)
    valid = (t[None, :] >= vmin[:, None]) & (t[None, :] < vmax[:, None])
    var12 = np.where(valid, var12, np.float32(-1.0))
    th = np.argmax(var12, axis=1)
    th = np.where(th == 0, 1, th)
    th = np.where(th == 255, 254, th)
    bad = vmin == vmax
    return th, bad


def _min_x_for_cut(c):
    """Smallest fp32 x with fl(255*x) >= c (c integer 1..255)."""
    f255 = np.float32(255.0)
    d = np.float32(np.float64(c) / 255.0)
    # walk down while still satisfying, then ensure satisfied
    for _ in range(8):
        dn = np.nextafter(d, np.float32(-1.0), dtype=np.float32)
        if np.float32(f255 * dn) >= c:
            d = dn
        else:
            break
    while np.float32(f255 * d) < c:
        d = np.nextafter(d, np.float32(2.0), dtype=np.float32)
    return d


_CUT_TABLE = None


def cut_table():
    global _CUT_TABLE
    if _CUT_TABLE is None:
        _CUT_TABLE = np.array(
            [np.float32(0.0)] + [_min_x_for_cut(c) for c in range(1, 256)],
            dtype=np.float32,
        )
    return _CUT_TABLE


def thresholds_to_cuts(th, bad):
    """mask = (cam > th) == (x >= dth); degenerate images -> never."""
    tab = cut_table()
    dth = tab[np.asarray(th) + 1]
    return np.where(bad, np.float32(2.0), dth).astype(np.float32)


# ---------------------------------------------------------------------------
_NC_CACHE = {}


def _get_ncs():
    if "hist" not in _NC_CACHE:
        _NC_CACHE["hist"] = build_hist_nc()
        _NC_CACHE["mask"] = build_mask_nc()
    return _NC_CACHE["hist"], _NC_CACHE["mask"]


def kernel(x: np.ndarray, _profile: dict | None = None) -> np.ndarray:
    x = np.ascontiguousarray(np.asarray(x, dtype=np.float32))
    assert x.shape == (B, N, H, W_IMG)
    nc_hist, nc_mask = _get_ncs()

    bpc = B // N_CORES
    # partition-major [128, nimg, cpi] shards: one fat contiguous DMA run per
    # partition line instead of 1.5KB descriptors
    shards = [
        np.ascontiguousarray(
            x[k * bpc:(k + 1) * bpc]
            .reshape(IMGS_PER_CORE, PARTS, CPI)
            .transpose(1, 0, 2)
        )
        for k in range(N_CORES)
    ]
    core_ids = list(range(N_CORES))

    kwargs_a = dict(_profile.get("a", {})) if _profile else {}
    res_a = run_bass_kernel_spmd(
        nc_hist, [{"x": s} for s in shards], core_ids=core_ids, **kwargs_a
    )
    if _profile is not None:
        _profile["res_a"] = res_a

    dths = []
    for k in range(N_CORES):
        hist = decode_hist(res_a.results[k]["w_raw"])
        th, bad = otsu_f32(hist)
        dth = thresholds_to_cuts(th, bad)
        dths.append(np.repeat(dth[:, None], PARTS, axis=1).astype(np.float32))

    kwargs_b = dict(_profile.get("b", {})) if _profile else {}
    res_b = run_bass_kernel_spmd(
        nc_mask,
        [{"x": s, "dth": d} for s, d in zip(shards, dths)],
        core_ids=core_ids,
        **kwargs_b,
    )
    if _profile is not None:
        _profile["res_b"] = res_b

    out = np.empty((B, N, H, W_IMG), np.int32)
    for k in range(N_CORES):
        m = res_b.results[k]["mask"]  # [128, 64, 392] bf16 (1.0/0.0)
        mi = (np.asarray(m).view(np.uint16) != 0).astype(np.int32)
        out[k * bpc:(k + 1) * bpc] = (
            mi.transpose(1, 0, 2).reshape(bpc, N, H, W_IMG)
        )
    return out



# revision 7
# speedup vs baseline: 1.1022x; 1.1022x over previous
"""Otsu-threshold binary region proposal kernel for Trainium2 (8 NeuronCores).

Algorithm (per image of 224*224 pixels, 512 images total, data-parallel over
8 cores / 64 images per core):

  reference:  cam = floor(x*255); per-image 256-bin histogram; Otsu threshold
              via argmax of inter-class variance restricted to [vmin, vmax);
              roi = (cam > th), 0 for degenerate images.

Device pass A (histogram + cam cache):
  Thermometer decomposition: with hi = cam >> 4, lo = cam & 15,
      R[tau, sigma] = sum_p colA_tau(p) * colB_sigma(p)
  colA_tau ~ [cam >= 16*tau] (pack-major, stationary), colB_sigma ~ [lo >=
  sigma] (cut-major, moving); 16x16 pair counts accumulate on the PE as one
  [128,128]x[128,128] matmul per 8-chunk pack (block-diagonal; host sums the
  c'==c'' blocks).  Engine split is tuned so DVE / ACT / GPSIMD all sit near
  the PE's ~210us roofline: DVE takes the fused floor prep (255x + magic in
  2 ops), the contiguous B rows (4x mode) and a few strided A rows (2x);
  ACT takes the remaining A rows (Sign, +-1 coded); GPSIMD takes the memset
  ones-rows and a slice of contiguous B rows.  Pass A also emits cam as
  uint8 (ci8) to DRAM so pass B never re-reads x.

Host (exact float32, mirrors jax reference op-for-op):
  decode W -> 256-bin histogram -> cumsums -> inter-class variance -> argmax
  -> th.  mask = (cam > th) == (ci8 >= th+1), exact in integers: no float
  cut table needed.  Degenerate images get th+1 = 256 (never fires).

Device pass B (mask): mask = (ci8 >= thp1) as uint8; reads 3.2MB instead of
the 12.8MB of x, writes 3.2MB.

floor() trick (no floor ALU op): t = fl(255x + (2^23 - 0.5)); ci = t - 2^23
= floor(fl(255x)) exactly (fp32 ulp 1.0 at 2^23 forces round-to-integer;
the -0.5 biases the tie so exact integers round down correctly... for
s = fl(255x) integer the only case is s=0 -> t = 2^23 - 0.5 rounds to 2^23
(ties-to-even) -> ci = 0 correct).
"""

import math
import os
import sys

import numpy as np

sys.path.insert(0, "/opt/trn_rl_repo")

import concourse.bacc as bacc
import concourse.bass as bass  # noqa: F401
import concourse.mybir as mybir
from concourse.bass_utils import run_bass_kernel_spmd
from concourse.tile import TileContext

# ---------------------------------------------------------------------------
# Problem geometry (hardcoded per spec)
B, N, H, W_IMG = 64, 8, 224, 224
PIX = H * W_IMG              # 50176
PARTS = 128
CPI = PIX // PARTS           # 392 chunks (columns) per image
N_CORES = 8
IMGS_PER_CORE = (B // N_CORES) * N      # 64
NBINS = 256

# Tunables -----------------------------------------------------------------
GROUP = 3          # images per thermo group
# Probe-measured (FD2048, ns): DVE is_ge bf16-out ~680 both contiguous AND
# [1,8]-run strided (4x either way); DVE u8/fp32 ops ~1210 (2x_2p); ACT ~1150
# per col; GPSIMD tensor_scalar ~31800 (15.5ns/elem — NEVER use); GPSIMD
# memset ~100 (free).  So feature columns go DVE-first, ACT takes the
# overflow, GPSIMD only memsets.
# A-plane (pack-major, stationary): rows 1..A_DVE_ROWS on DVE, rows
# A_DVE_ROWS+1..15 on ACT (Sign, +-1 coded).  B-plane (cut-major, moving):
# rows 1..15-B_ACT_ROWS on DVE, last B_ACT_ROWS on ACT (Sign, +-1).
A_DVE_ROWS = 8
B_ACT_ROWS = 1
MGROUP = 16        # images per pass-B tile group

FP32 = mybir.dt.float32
BF16 = mybir.dt.bfloat16
I16 = mybir.dt.int16
U8 = mybir.dt.uint8
ALU = mybir.AluOpType
ACTF = mybir.ActivationFunctionType
MAGIC = 8388608.0  # 2^23


def _enc_pm():
    """Which rows are +-1 coded (ACT Sign)."""
    encA = np.zeros(16, dtype=bool)
    encB = np.zeros(16, dtype=bool)
    encA[A_DVE_ROWS + 1:] = True
    if B_ACT_ROWS:
        encB[16 - B_ACT_ROWS:] = True
    return encA, encB


# ---------------------------------------------------------------------------
# Pass A: histogram + ci8 cache
def build_hist_nc(nimg=IMGS_PER_CORE, cpi=CPI, group=GROUP):
    assert cpi % 8 == 0
    nc = bacc.Bacc("TRN2", target_bir_lowering=False, debug=False)
    x_d = nc.dram_tensor("x", [PARTS, nimg, cpi], FP32, kind="ExternalInput")
    w_d = nc.dram_tensor("w_raw", [nimg, PARTS, PARTS], FP32, kind="ExternalOutput")
    c8_d = nc.dram_tensor("ci8", [PARTS, nimg, cpi], U8, kind="ExternalOutput")

    with TileContext(nc) as tc:
        with (
            tc.tile_pool(name="const", bufs=1) as cpool,
            tc.tile_pool(name="xin", bufs=2) as xpool,
            tc.tile_pool(name="prep", bufs=2) as ppool,
            tc.tile_pool(name="thermo", bufs=2) as tpool,
            tc.tile_pool(name="psum", bufs=4, space="PSUM") as qpool,
        ):
            # per-partition biases for ACT Sign rows: sign(v - cut + 0.5)
            nACT = (15 - A_DVE_ROWS) + B_ACT_ROWS
            act_bias = cpool.tile([PARTS, max(nACT, 1)], FP32, tag="abias")
            ab_idx = {}
            j = 0
            for tau in range(A_DVE_ROWS + 1, 16):
                nc.vector.memset(act_bias[:, j:j + 1], 0.5 - 16 * tau)
                ab_idx[("A", tau)] = j
                j += 1
            for sg in range(16 - B_ACT_ROWS, 16):
                nc.vector.memset(act_bias[:, j:j + 1], 0.5 - sg)
                ab_idx[("B", sg)] = j
                j += 1

            n_groups = math.ceil(nimg / group)
            for g in range(n_groups):
                g0 = g * group
                g1 = min(g0 + group, nimg)
                gw = (g1 - g0) * cpi
                gw8 = gw // 8

                x_t = xpool.tile([PARTS, group * cpi], FP32, tag="x")
                ci = ppool.tile([PARTS, group * cpi], I16, tag="ci")
                ci8 = ppool.tile([PARTS, group * cpi], U8, tag="ci8")
                lo = ppool.tile([PARTS, group * cpi], I16, tag="lo")
                A_t = tpool.tile([PARTS, group * cpi // 8, 16, 8], BF16, tag="A")
                B_t = tpool.tile([PARTS, 16, group * cpi], BF16, tag="B")

                nc.sync.dma_start(
                    out=x_t[:, :gw],
                    in_=bass.AP(
                        x_d, g0 * cpi,
                        [[nimg * cpi, PARTS], [1, gw]],
                    ),
                )

                # floor prep, all on DVE.  CAUTION: DVE op0/op1 chains are
                # FUSED (no intermediate fp32 rounding — probe-verified), so
                # the 255* multiply must be a LONE op to round fl(255x)
                # before the magic add; the add-add chain itself is safe
                # fused (s - 0.5 exact, convert rounds to nearest) or
                # unfused (classic 2^23 magic).
                nc.vector.tensor_scalar(
                    out=x_t[:, :gw], in0=x_t[:, :gw],
                    scalar1=255.0, scalar2=None, op0=ALU.mult,
                )
                nc.vector.tensor_scalar(
                    out=ci[:, :gw], in0=x_t[:, :gw],
                    scalar1=MAGIC - 0.5, scalar2=-MAGIC,
                    op0=ALU.add, op1=ALU.add,
                )
                nc.vector.tensor_scalar(
                    out=ci8[:, :gw], in0=x_t[:, :gw],
                    scalar1=MAGIC - 0.5, scalar2=-MAGIC,
                    op0=ALU.add, op1=ALU.add,
                )
                nc.sync.dma_start(
                    out=bass.AP(
                        c8_d, g0 * cpi,
                        [[nimg * cpi, PARTS], [1, gw]],
                    ),
                    in_=ci8[:, :gw],
                )
                nc.vector.tensor_scalar(
                    out=lo[:, :gw], in0=ci[:, :gw],
                    scalar1=15, scalar2=None, op0=ALU.bitwise_and,
                )

                ci_v = ci[:, :gw].rearrange("p (a b) -> p a b", b=8)
                # A plane (pack-major): memset + DVE strided + ACT Sign
                nc.gpsimd.memset(A_t[:, :gw8, 0, :], 1.0)
                for tau in range(1, A_DVE_ROWS + 1):
                    nc.vector.tensor_scalar(
                        out=A_t[:, :gw8, tau, :], in0=ci_v,
                        scalar1=16 * tau, scalar2=None, op0=ALU.is_ge,
                    )
                for tau in range(A_DVE_ROWS + 1, 16):
                    j = ab_idx[("A", tau)]
                    nc.scalar.activation(
                        out=A_t[:, :gw8, tau, :], in_=ci_v,
                        func=ACTF.Sign,
                        bias=act_bias[:, j:j + 1],
                        scale=1.0,
                    )
                # B plane (cut-major): memset + DVE 4x + ACT tail
                nc.gpsimd.memset(B_t[:, 0, :gw], 1.0)
                for sg in range(1, 16 - B_ACT_ROWS):
                    nc.vector.tensor_scalar(
                        out=B_t[:, sg, :gw], in0=lo[:, :gw],
                        scalar1=sg, scalar2=None, op0=ALU.is_ge,
                    )
                for sg in range(16 - B_ACT_ROWS, 16):
                    j = ab_idx[("B", sg)]
                    nc.scalar.activation(
                        out=B_t[:, sg, :gw], in_=lo[:, :gw],
                        func=ACTF.Sign,
                        bias=act_bias[:, j:j + 1],
                        scale=1.0,
                    )

                # PE: per image, 49 packed [128,128] matmuls accumulate in PSUM
                packs_per_img = cpi // 8
                for i in range(g0, g1):
                    il = i - g0
                    psum_t = qpool.tile([PARTS, PARTS], FP32, tag="ps")
                    for k in range(packs_per_img):
                        p = il * packs_per_img + k
                        nc.tensor.matmul(
                            psum_t[:],
                            A_t[:, p, :, :].rearrange("p a b -> p (a b)"),
                            B_t[:, :, 8 * p:8 * p + 8],
                            start=(k == 0),
                            stop=(k == packs_per_img - 1),
                        )
                    w_sb = ppool.tile([PARTS, PARTS], FP32, tag="wsb")
                    if i % 3 == 0:
                        nc.vector.tensor_copy(out=w_sb[:], in_=psum_t[:])
                    else:
                        nc.scalar.copy(w_sb[:], psum_t[:])
                    nc.sync.dma_start(out=w_d.ap()[i], in_=w_sb[:])
    nc.finalize()
    return nc


# ---------------------------------------------------------------------------
# Pass B: mask from cached ci8
def build_mask_nc(nimg=IMGS_PER_CORE, cpi=CPI, mgroup=MGROUP):
    nc = bacc.Bacc("TRN2", target_bir_lowering=False, debug=False)
    c8_d = nc.dram_tensor("ci8", [PARTS, nimg, cpi], U8, kind="ExternalInput")
    t_d = nc.dram_tensor("thp1", [PARTS, nimg], FP32, kind="ExternalInput")
    m_d = nc.dram_tensor("mask", [PARTS, nimg, cpi], U8, kind="ExternalOutput")

    with TileContext(nc) as tc:
        with (
            tc.tile_pool(name="cst", bufs=1) as cpool,
            tc.tile_pool(name="cin", bufs=3) as xpool,
            tc.tile_pool(name="mo", bufs=3) as mpool,
        ):
            th_all = cpool.tile([PARTS, nimg], FP32, tag="t")
            nc.sync.dma_start(out=th_all[:], in_=t_d.ap())
            for g0 in range(0, nimg, mgroup):
                g1 = min(g0 + mgroup, nimg)
                gl = g1 - g0
                c_t = xpool.tile([PARTS, mgroup * cpi], U8, tag="c")
                m_t = mpool.tile([PARTS, mgroup * cpi], U8, tag="m")
                nc.sync.dma_start(
                    out=c_t[:, :gl * cpi],
                    in_=bass.AP(
                        c8_d, g0 * cpi,
                        [[nimg * cpi, PARTS], [1, gl * cpi]],
                    ),
                )
                for i in range(g0, g1):
                    il = i - g0
                    nc.vector.tensor_scalar(
                        out=m_t[:, il * cpi:(il + 1) * cpi],
                        in0=c_t[:, il * cpi:(il + 1) * cpi],
                        scalar1=th_all[:, i:i + 1],
                        scalar2=None, op0=ALU.is_ge,
                    )
                nc.sync.dma_start(
                    out=bass.AP(
                        m_d, g0 * cpi,
                        [[nimg * cpi, PARTS], [1, gl * cpi]],
                    ),
                    in_=m_t[:, :gl * cpi],
                )
    nc.finalize()
    return nc


# ---------------------------------------------------------------------------
# Host: decode W, exact-float32 Otsu
def decode_hist(w_raw, nimg=IMGS_PER_CORE, npix=PIX):
    """w_raw [nimg, 128, 128] fp32 -> hist [nimg, 256] int64 (exact).

    Psum row 8*tau+c', col 8*sigma+c'': sum the c'==c'' diagonal blocks."""
    encA, encB = _enc_pm()
    P128 = np.round(np.asarray(w_raw, np.float64)).astype(np.int64)
    P128 = P128.reshape(nimg, 16, 8, 16, 8)  # [img, tau, c', sigma, c'']
    R = np.einsum("itcsc->its", P128)        # [img, tau, sigma]
    P = npix
    sumB = np.where(encB[None, :], (R[:, 0, :] + P) // 2, R[:, 0, :])
    sumA = np.where(encA[None, :], (R[:, :, 0] + P) // 2, R[:, :, 0])
    eA = encA[None, :, None]
    eB = encB[None, None, :]
    sA = sumA[:, :, None]
    sB = sumB[:, None, :]
    W = np.where(
        ~eA & ~eB, R,
        np.where(
            eA & ~eB, (R + sB) // 2,
            np.where(~eA & eB, (R + sA) // 2, (R + 2 * sA + 2 * sB - P) // 4),
        ),
    )
    chk = np.where(
        ~eA & ~eB, 0,
        np.where(eA & ~eB, (R + sB) % 2,
                 np.where(~eA & eB, (R + sA) % 2, (R + 2 * sA + 2 * sB - P) % 4)),
    )
    assert not chk.any(), "non-integer decode: device histogram corrupted"
    Wp = np.zeros((nimg, 17, 17), np.int64)
    Wp[:, :16, :16] = W
    hist = (Wp[:, :16, :16] - Wp[:, 1:, :16] - Wp[:, :16, 1:] + Wp[:, 1:, 1:])
    hist = hist.reshape(nimg, 256)
    assert (hist >= 0).all() and (hist.sum(1) == P).all(), "bad histogram"
    return hist


def otsu_f32(hist):
    """Mirror the jax float32 reference exactly. hist [n,256] int64 -> th, bad."""
    f = hist.astype(np.float32)
    centers = np.arange(NBINS, dtype=np.float32)
    w1 = np.cumsum(f, axis=1, dtype=np.float32)
    total = w1[:, -1:]
    s1 = np.cumsum(f * centers, axis=1, dtype=np.float32)
    stot = s1[:, -1:]
    w2 = total - w1
    with np.errstate(divide="ignore", invalid="ignore"):
        m1 = s1 / w1
        m2 = (stot - s1) / w2
        d = m1 - m2
        var12 = (w1 * w2) * (d * d)
    nz = hist > 0
    t = np.arange(NBINS)
    vmin = np.argmax(nz, axis=1)
    vmax = NBINS - 1 - np.argmax(nz[:, ::-1], axis=1)
    valid = (t[None, :] >= vmin[:, None]) & (t[None, :] < vmax[:, None])
    var12 = np.where(valid, var12, np.float32(-1.0))
    th = np.argmax(var12, axis=1)
    th = np.where(th == 0, 1, th)
    th = np.where(th == 255, 254, th)
    bad = vmin == vmax
    return th, bad


# ---------------------------------------------------------------------------
_NC_CACHE = {}


def _get_ncs():
    if "hist" not in _NC_CACHE:
        _NC_CACHE["hist"] = build_hist_nc()
        _NC_CACHE["mask"] = build_mask_nc()
    return _NC_CACHE["hist"], _NC_CACHE["mask"]


def kernel(x: np.ndarray, _profile: dict | None = None) -> np.ndarray:
    x = np.ascontiguousarray(np.asarray(x, dtype=np.float32))
    assert x.shape == (B, N, H, W_IMG)
    nc_hist, nc_mask = _get_ncs()

    bpc = B // N_CORES
    shards = [
        np.ascontiguousarray(
            x[k * bpc:(k + 1) * bpc]
            .reshape(IMGS_PER_CORE, PARTS, CPI)
            .transpose(1, 0, 2)
        )
        for k in range(N_CORES)
    ]
    core_ids = list(range(N_CORES))

    kwargs_a = dict(_profile.get("a", {})) if _profile else {}
    res_a = run_bass_kernel_spmd(
        nc_hist, [{"x": s} for s in shards], core_ids=core_ids, **kwargs_a
    )
    if _profile is not None:
        _profile["res_a"] = res_a

    thp1s = []
    for k in range(N_CORES):
        hist = decode_hist(res_a.results[k]["w_raw"])
        th, bad = otsu_f32(hist)
        thp1 = np.where(bad, np.float32(256.0), (th + 1).astype(np.float32))
        thp1s.append(
            np.ascontiguousarray(
                np.broadcast_to(thp1[None, :], (PARTS, IMGS_PER_CORE))
            ).astype(np.float32)
        )

    kwargs_b = dict(_profile.get("b", {})) if _profile else {}
    res_b = run_bass_kernel_spmd(
        nc_mask,
        [{"ci8": np.asarray(res_a.results[k]["ci8"]), "thp1": thp1s[k]}
         for k in range(N_CORES)],
        core_ids=core_ids,
        **kwargs_b,
    )
    if _profile is not None:
        _profile["res_b"] = res_b

    out = np.empty((B, N, H, W_IMG), np.int32)
    for k in range(N_CORES):
        m = np.asarray(res_b.results[k]["mask"])  # [128, 64, 392] u8
        out[k * bpc:(k + 1) * bpc] = (
            m.astype(np.int32).transpose(1, 0, 2).reshape(bpc, N, H, W_IMG)
        )
    return out


# revision 11
# speedup vs baseline: 1.1304x; 1.0255x over previous
"""Otsu-threshold binary region proposal kernel for Trainium2 (8 NeuronCores).

Algorithm (per image of 224*224 pixels, 512 images total, data-parallel over
8 cores / 64 images per core):

  reference:  cam = floor(x*255); per-image 256-bin histogram; Otsu threshold
              via argmax of inter-class variance restricted to [vmin, vmax);
              roi = (cam > th), 0 for degenerate images.

Device pass A (histogram + cam cache):
  Thermometer decomposition: with hi = cam >> 4, lo = cam & 15,
      R[tau, sigma] = sum_p colA_tau(p) * colB_sigma(p)
  colA_tau ~ [cam >= 16*tau] (pack-major, stationary), colB_sigma ~ [lo >=
  sigma] (cut-major, moving); 16x16 pair counts accumulate on the PE as one
  [128,128]x[128,128] matmul per 8-chunk pack (block-diagonal; host sums the
  c'==c'' blocks).  Engine split is tuned so DVE / ACT / GPSIMD all sit near
  the PE's ~210us roofline: DVE takes the fused floor prep (255x + magic in
  2 ops), the contiguous B rows (4x mode) and a few strided A rows (2x);
  ACT takes the remaining A rows (Sign, +-1 coded); GPSIMD takes the memset
  ones-rows and a slice of contiguous B rows.  Pass A also emits cam as
  uint8 (ci8) to DRAM so pass B never re-reads x.

Host (exact float32, mirrors jax reference op-for-op):
  decode W -> 256-bin histogram -> cumsums -> inter-class variance -> argmax
  -> th.  mask = (cam > th) == (ci8 >= th+1), exact in integers: no float
  cut table needed.  Degenerate images get th+1 = 256 (never fires).

Device pass B (mask): mask = (ci8 >= thp1) as uint8; reads 3.2MB instead of
the 12.8MB of x, writes 3.2MB.

floor() trick (no floor ALU op): t = fl(255x + (2^23 - 0.5)); ci = t - 2^23
= floor(fl(255x)) exactly (fp32 ulp 1.0 at 2^23 forces round-to-integer;
the -0.5 biases the tie so exact integers round down correctly... for
s = fl(255x) integer the only case is s=0 -> t = 2^23 - 0.5 rounds to 2^23
(ties-to-even) -> ci = 0 correct).
"""

import math
import os
import sys

import numpy as np

sys.path.insert(0, "/opt/trn_rl_repo")

import concourse.bacc as bacc
import concourse.bass as bass  # noqa: F401
import concourse.mybir as mybir
from concourse.bass_utils import run_bass_kernel_spmd
from concourse.tile import TileContext

# ---------------------------------------------------------------------------
# Problem geometry (hardcoded per spec)
B, N, H, W_IMG = 64, 8, 224, 224
PIX = H * W_IMG              # 50176
PARTS = 128
CPI = PIX // PARTS           # 392 chunks (columns) per image
N_CORES = 8
IMGS_PER_CORE = (B // N_CORES) * N      # 64
NBINS = 256

# Tunables -----------------------------------------------------------------
GROUP = 3          # images per thermo group
# Probe-measured (ns, FD1176-equivalent): DVE is_ge bf16-out ~457 at 4x
# (contiguous OR [1,8]-run strided) but degrades to ~755 (2x) when ANOTHER
# engine is concurrently writing the same SBUF tile (port contention — same
# mechanism the GPSIMD note below describes).  DVE u8/fp32 ops ~755 (2x_2p);
# ACT ~1260 per col (1x always); GPSIMD tensor_scalar ~15.5ns/elem (NEVER
# use); GPSIMD memset ~free.  So: feature columns go DVE-first, ACT takes
# the A-row overflow, GPSIMD only memsets, and emission order staggers the
# writers of each tile (GP memsets, then ACT's Sign rows, then DVE's rows
# after filler work) to keep single-writer-per-tile windows.
# A-plane (pack-major, stationary): rows 1..A_DVE_ROWS on DVE, rows
# A_DVE_ROWS+1..15 on ACT (Sign, +-1 coded).  B-plane (cut-major, moving):
# all rows 1..15 on DVE with the lo-AND folded into each op (and,is_ge
# chain); ci8 conversion on ACT.
A_DVE_ROWS = 7
B_ACT_ROWS = 0
MGROUP = 16        # images per pass-B tile group

FP32 = mybir.dt.float32
BF16 = mybir.dt.bfloat16
I16 = mybir.dt.int16
U8 = mybir.dt.uint8
ALU = mybir.AluOpType
ACTF = mybir.ActivationFunctionType
MAGIC = 8388608.0  # 2^23


def _enc_pm():
    """Which rows are +-1 coded (ACT Sign)."""
    encA = np.zeros(16, dtype=bool)
    encB = np.zeros(16, dtype=bool)
    encA[A_DVE_ROWS + 1:] = True
    if B_ACT_ROWS:
        encB[16 - B_ACT_ROWS:] = True
    return encA, encB


# ---------------------------------------------------------------------------
# Pass A: histogram + ci8 cache
def build_hist_nc(nimg=IMGS_PER_CORE, cpi=CPI, group=GROUP):
    assert cpi % 8 == 0
    nc = bacc.Bacc("TRN2", target_bir_lowering=False, debug=False)
    x_d = nc.dram_tensor("x", [PARTS, nimg, cpi], FP32, kind="ExternalInput")
    w_d = nc.dram_tensor("w_raw", [nimg, PARTS, PARTS], FP32, kind="ExternalOutput")
    c8_d = nc.dram_tensor("ci8", [PARTS, nimg, cpi], U8, kind="ExternalOutput")

    with TileContext(nc) as tc:
        with (
            tc.tile_pool(name="const", bufs=1) as cpool,
            tc.tile_pool(name="xin", bufs=2) as xpool,
            tc.tile_pool(name="prep", bufs=2) as ppool,
            tc.tile_pool(name="thermo", bufs=2) as tpool,
            tc.tile_pool(name="psum", bufs=4, space="PSUM") as qpool,
        ):
            # per-partition biases for ACT Sign rows: sign(v - cut + 0.5)
            nACT = 15 - A_DVE_ROWS
            act_bias = cpool.tile([PARTS, max(nACT, 1)], FP32, tag="abias")
            ab_idx = {}
            j = 0
            for tau in range(A_DVE_ROWS + 1, 16):
                nc.vector.memset(act_bias[:, j:j + 1], 0.5 - 16 * tau)
                ab_idx[("A", tau)] = j
                j += 1

            n_groups = math.ceil(nimg / group)

            def emit_load_prep(g):
                """DMA x(g) in, then DVE floor prep.  CAUTION: DVE op0/op1
                chains are FUSED (no intermediate fp32 rounding — probe-
                verified), so the 255* multiply must be a LONE op to round
                fl(255x) before the magic add; the add-add chain itself is
                safe fused (s-0.5 exact, convert rounds nearest) or unfused
                (classic 2^23 magic)."""
                g0 = g * group
                g1 = min(g0 + group, nimg)
                gw = (g1 - g0) * cpi
                x_t = xpool.tile([PARTS, group * cpi], FP32, tag="x")
                ci = ppool.tile([PARTS, group * cpi], I16, tag="ci")
                nc.sync.dma_start(
                    out=x_t[:, :gw],
                    in_=bass.AP(x_d, g0 * cpi, [[nimg * cpi, PARTS], [1, gw]]),
                )
                nc.vector.tensor_scalar(
                    out=x_t[:, :gw], in0=x_t[:, :gw],
                    scalar1=255.0, scalar2=None, op0=ALU.mult,
                )
                nc.vector.tensor_scalar(
                    out=ci[:, :gw], in0=x_t[:, :gw],
                    scalar1=MAGIC - 0.5, scalar2=-MAGIC,
                    op0=ALU.add, op1=ALU.add,
                )
                lo = ppool.tile([PARTS, group * cpi], I16, tag="lo")
                nc.vector.tensor_scalar(
                    out=lo[:, :gw], in0=ci[:, :gw],
                    scalar1=15, scalar2=None, op0=ALU.bitwise_and,
                )
                return ci, lo

            cur = emit_load_prep(0)
            for g in range(n_groups):
                g0 = g * group
                g1 = min(g0 + group, nimg)
                gw = (g1 - g0) * cpi
                gw8 = gw // 8
                ci, lo = cur

                ci8 = ppool.tile([PARTS, group * cpi], U8, tag="ci8")
                A_t = tpool.tile([PARTS, group * cpi // 8, 16, 8], BF16, tag="A")
                B_t = tpool.tile([PARTS, 16, group * cpi], BF16, tag="B")

                # ones rows first (GPSIMD, ~free, before other writers)
                nc.gpsimd.memset(A_t[:, :gw8, 0, :], 1.0)
                nc.gpsimd.memset(B_t[:, 0, :gw], 1.0)

                ci_v = ci[:, :gw].rearrange("p (a b) -> p a b", b=8)
                # ACT: Sign rows of the A plane + the ci8 u8 conversion.
                # Emitted before DVE's blocks so ACT is done with the A tile
                # by the time DVE's A rows issue (single-writer windows).
                for tau in range(A_DVE_ROWS + 1, 16):
                    j = ab_idx[("A", tau)]
                    nc.scalar.activation(
                        out=A_t[:, :gw8, tau, :], in_=ci_v,
                        func=ACTF.Sign,
                        bias=act_bias[:, j:j + 1],
                        scale=1.0,
                    )
                nc.scalar.activation(
                    out=ci8[:, :gw], in_=ci[:, :gw],
                    func=ACTF.Copy, bias=0.0, scale=1.0,
                )
                nc.sync.dma_start(
                    out=bass.AP(c8_d, g0 * cpi, [[nimg * cpi, PARTS], [1, gw]]),
                    in_=ci8[:, :gw],
                )

                # DVE: B plane rows
                for sg in range(1, 16):
                    nc.vector.tensor_scalar(
                        out=B_t[:, sg, :gw], in0=lo[:, :gw],
                        scalar1=sg, scalar2=None, op0=ALU.is_ge,
                    )

                # filler between DVE's B and A blocks: next group's load+prep
                if g + 1 < n_groups:
                    cur = emit_load_prep(g + 1)

                # DVE: A plane rows (ACT has finished its A rows by now)
                for tau in range(1, A_DVE_ROWS + 1):
                    nc.vector.tensor_scalar(
                        out=A_t[:, :gw8, tau, :], in0=ci_v,
                        scalar1=16 * tau, scalar2=None, op0=ALU.is_ge,
                    )

                # PE: per image, 49 packed [128,128] matmuls accumulate in PSUM
                packs_per_img = cpi // 8
                for i in range(g0, g1):
                    il = i - g0
                    psum_t = qpool.tile([PARTS, PARTS], FP32, tag="ps")
                    for k in range(packs_per_img):
                        p = il * packs_per_img + k
                        nc.tensor.matmul(
                            psum_t[:],
                            A_t[:, p, :, :].rearrange("p a b -> p (a b)"),
                            B_t[:, :, 8 * p:8 * p + 8],
                            start=(k == 0),
                            stop=(k == packs_per_img - 1),
                        )
                    w_sb = ppool.tile([PARTS, PARTS], FP32, tag="wsb")
                    if i % 3 == 0:
                        nc.vector.tensor_copy(out=w_sb[:], in_=psum_t[:])
                    else:
                        nc.scalar.copy(w_sb[:], psum_t[:])
                    nc.sync.dma_start(out=w_d.ap()[i], in_=w_sb[:])
    nc.finalize()
    return nc


# ---------------------------------------------------------------------------
# Pass B: mask from cached ci8
def build_mask_nc(nimg=IMGS_PER_CORE, cpi=CPI, mgroup=MGROUP):
    nc = bacc.Bacc("TRN2", target_bir_lowering=False, debug=False)
    c8_d = nc.dram_tensor("ci8", [PARTS, nimg, cpi], U8, kind="ExternalInput")
    t_d = nc.dram_tensor("thp1", [PARTS, nimg], FP32, kind="ExternalInput")
    m_d = nc.dram_tensor("mask", [PARTS, nimg, cpi], U8, kind="ExternalOutput")

    with TileContext(nc) as tc:
        with (
            tc.tile_pool(name="cst", bufs=1) as cpool,
            tc.tile_pool(name="cin", bufs=3) as xpool,
            tc.tile_pool(name="mo", bufs=3) as mpool,
        ):
            th_all = cpool.tile([PARTS, nimg], FP32, tag="t")
            nc.sync.dma_start(out=th_all[:], in_=t_d.ap())
            for g0 in range(0, nimg, mgroup):
                g1 = min(g0 + mgroup, nimg)
                gl = g1 - g0
                c_t = xpool.tile([PARTS, mgroup * cpi], U8, tag="c")
                m_t = mpool.tile([PARTS, mgroup * cpi], U8, tag="m")
                nc.sync.dma_start(
                    out=c_t[:, :gl * cpi],
                    in_=bass.AP(
                        c8_d, g0 * cpi,
                        [[nimg * cpi, PARTS], [1, gl * cpi]],
                    ),
                )
                for i in range(g0, g1):
                    il = i - g0
                    nc.vector.tensor_scalar(
                        out=m_t[:, il * cpi:(il + 1) * cpi],
                        in0=c_t[:, il * cpi:(il + 1) * cpi],
                        scalar1=th_all[:, i:i + 1],
                        scalar2=None, op0=ALU.is_ge,
                    )
                nc.sync.dma_start(
                    out=bass.AP(
                        m_d, g0 * cpi,
                        [[nimg * cpi, PARTS], [1, gl * cpi]],
                    ),
                    in_=m_t[:, :gl * cpi],
                )
    nc.finalize()
    return nc


# ---------------------------------------------------------------------------
# Host: decode W, exact-float32 Otsu
def decode_hist(w_raw, nimg=IMGS_PER_CORE, npix=PIX):
    """w_raw [nimg, 128, 128] fp32 -> hist [nimg, 256] int64 (exact).

    Psum row 8*tau+c', col 8*sigma+c'': sum the c'==c'' diagonal blocks."""
    encA, encB = _enc_pm()
    P128 = np.round(np.asarray(w_raw, np.float64)).astype(np.int64)
    P128 = P128.reshape(nimg, 16, 8, 16, 8)  # [img, tau, c', sigma, c'']
    R = np.einsum("itcsc->its", P128)        # [img, tau, sigma]
    P = npix
    sumB = np.where(encB[None, :], (R[:, 0, :] + P) // 2, R[:, 0, :])
    sumA = np.where(encA[None, :], (R[:, :, 0] + P) // 2, R[:, :, 0])
    eA = encA[None, :, None]
    eB = encB[None, None, :]
    sA = sumA[:, :, None]
    sB = sumB[:, None, :]
    W = np.where(
        ~eA & ~eB, R,
        np.where(
            eA & ~eB, (R + sB) // 2,
            np.where(~eA & eB, (R + sA) // 2, (R + 2 * sA + 2 * sB - P) // 4),
        ),
    )
    chk = np.where(
        ~eA & ~eB, 0,
        np.where(eA & ~eB, (R + sB) % 2,
                 np.where(~eA & eB, (R + sA) % 2, (R + 2 * sA + 2 * sB - P) % 4)),
    )
    assert not chk.any(), "non-integer decode: device histogram corrupted"
    Wp = np.zeros((nimg, 17, 17), np.int64)
    Wp[:, :16, :16] = W
    hist = (Wp[:, :16, :16] - Wp[:, 1:, :16] - Wp[:, :16, 1:] + Wp[:, 1:, 1:])
    hist = hist.reshape(nimg, 256)
    assert (hist >= 0).all() and (hist.sum(1) == P).all(), "bad histogram"
    return hist


def otsu_f32(hist):
    """Mirror the jax float32 reference exactly. hist [n,256] int64 -> th, bad."""
    f = hist.astype(np.float32)
    centers = np.arange(NBINS, dtype=np.float32)
    w1 = np.cumsum(f, axis=1, dtype=np.float32)
    total = w1[:, -1:]
    s1 = np.cumsum(f * centers, axis=1, dtype=np.float32)
    stot = s1[:, -1:]
    w2 = total - w1
    with np.errstate(divide="ignore", invalid="ignore"):
        m1 = s1 / w1
        m2 = (stot - s1) / w2
        d = m1 - m2
        var12 = (w1 * w2) * (d * d)
    nz = hist > 0
    t = np.arange(NBINS)
    vmin = np.argmax(nz, axis=1)
    vmax = NBINS - 1 - np.argmax(nz[:, ::-1], axis=1)
    valid = (t[None, :] >= vmin[:, None]) & (t[None, :] < vmax[:, None])
    var12 = np.where(valid, var12, np.float32(-1.0))
    th = np.argmax(var12, axis=1)
    th = np.where(th == 0, 1, th)
    th = np.where(th == 255, 254, th)
    bad = vmin == vmax
    return th, bad


# ---------------------------------------------------------------------------
_NC_CACHE = {}


def _get_ncs():
    if "hist" not in _NC_CACHE:
        _NC_CACHE["hist"] = build_hist_nc()
        _NC_CACHE["mask"] = build_mask_nc()
    return _NC_CACHE["hist"], _NC_CACHE["mask"]


def kernel(x: np.ndarray, _profile: dict | None = None) -> np.ndarray:
    x = np.ascontiguousarray(np.asarray(x, dtype=np.float32))
    assert x.shape == (B, N, H, W_IMG)
    nc_hist, nc_mask = _get_ncs()

    bpc = B // N_CORES
    shards = [
        np.ascontiguousarray(
            x[k * bpc:(k + 1) * bpc]
            .reshape(IMGS_PER_CORE, PARTS, CPI)
            .transpose(1, 0, 2)
        )
        for k in range(N_CORES)
    ]
    core_ids = list(range(N_CORES))

    kwargs_a = dict(_profile.get("a", {})) if _profile else {}
    res_a = run_bass_kernel_spmd(
        nc_hist, [{"x": s} for s in shards], core_ids=core_ids, **kwargs_a
    )
    if _profile is not None:
        _profile["res_a"] = res_a

    thp1s = []
    for k in range(N_CORES):
        hist = decode_hist(res_a.results[k]["w_raw"])
        th, bad = otsu_f32(hist)
        thp1 = np.where(bad, np.float32(256.0), (th + 1).astype(np.float32))
        thp1s.append(
            np.ascontiguousarray(
                np.broadcast_to(thp1[None, :], (PARTS, IMGS_PER_CORE))
            ).astype(np.float32)
        )

    kwargs_b = dict(_profile.get("b", {})) if _profile else {}
    res_b = run_bass_kernel_spmd(
        nc_mask,
        [{"ci8": np.asarray(res_a.results[k]["ci8"]), "thp1": thp1s[k]}
         for k in range(N_CORES)],
        core_ids=core_ids,
        **kwargs_b,
    )
    if _profile is not None:
        _profile["res_b"] = res_b

    out = np.empty((B, N, H, W_IMG), np.int32)
    for k in range(N_CORES):
        m = np.asarray(res_b.results[k]["mask"])  # [128, 64, 392] u8
        out[k * bpc:(k + 1) * bpc] = (
            m.astype(np.int32).transpose(1, 0, 2).reshape(bpc, N, H, W_IMG)
        )
    return out


# revision 13
# speedup vs baseline: 1.1390x; 1.0077x over previous
"""Otsu-threshold binary region proposal kernel for Trainium2 (8 NeuronCores).

Algorithm (per image of 224*224 pixels, 512 images total, data-parallel over
8 cores / 64 images per core):

  reference:  cam = floor(x*255); per-image 256-bin histogram; Otsu threshold
              via argmax of inter-class variance restricted to [vmin, vmax);
              roi = (cam > th), 0 for degenerate images.

Device pass A (histogram + cam cache):
  Thermometer decomposition: with hi = cam >> 4, lo = cam & 15,
      R[tau, sigma] = sum_p colA_tau(p) * colB_sigma(p)
  colA_tau ~ [cam >= 16*tau] (pack-major, stationary), colB_sigma ~ [lo >=
  sigma] (cut-major, moving); 16x16 pair counts accumulate on the PE as one
  [128,128]x[128,128] matmul per 8-chunk pack (block-diagonal; host sums the
  c'==c'' blocks).  Engine split is tuned so DVE / ACT / GPSIMD all sit near
  the PE's ~210us roofline: DVE takes the fused floor prep (255x + magic in
  2 ops), the contiguous B rows (4x mode) and a few strided A rows (2x);
  ACT takes the remaining A rows (Sign, +-1 coded); GPSIMD takes the memset
  ones-rows and a slice of contiguous B rows.  Pass A also emits cam as
  uint8 (ci8) to DRAM so pass B never re-reads x.

Host (exact float32, mirrors jax reference op-for-op):
  decode W -> 256-bin histogram -> cumsums -> inter-class variance -> argmax
  -> th.  mask = (cam > th) == (ci8 >= th+1), exact in integers: no float
  cut table needed.  Degenerate images get th+1 = 256 (never fires).

Device pass B (mask): mask = (ci8 >= thp1) as uint8; reads 3.2MB instead of
the 12.8MB of x, writes 3.2MB.

floor() trick (no floor ALU op): t = fl(255x + (2^23 - 0.5)); ci = t - 2^23
= floor(fl(255x)) exactly (fp32 ulp 1.0 at 2^23 forces round-to-integer;
the -0.5 biases the tie so exact integers round down correctly... for
s = fl(255x) integer the only case is s=0 -> t = 2^23 - 0.5 rounds to 2^23
(ties-to-even) -> ci = 0 correct).
"""

import math
import os
import sys

import numpy as np

sys.path.insert(0, "/opt/trn_rl_repo")

import concourse.bacc as bacc
import concourse.bass as bass  # noqa: F401
import concourse.mybir as mybir
from concourse.bass_utils import run_bass_kernel_spmd
from concourse.tile import TileContext

# ---------------------------------------------------------------------------
# Problem geometry (hardcoded per spec)
B, N, H, W_IMG = 64, 8, 224, 224
PIX = H * W_IMG              # 50176
PARTS = 128
CPI = PIX // PARTS           # 392 chunks (columns) per image
N_CORES = 8
IMGS_PER_CORE = (B // N_CORES) * N      # 64
NBINS = 256

# Tunables -----------------------------------------------------------------
GROUP = 3          # images per thermo group
# Probe-measured (ns, FD1176-equivalent): DVE is_ge bf16-out ~457 at 4x
# (contiguous OR [1,8]-run strided) but degrades to ~755 (2x) when ANOTHER
# engine is concurrently writing the same SBUF tile (port contention — same
# mechanism the GPSIMD note below describes).  DVE u8/fp32 ops ~755 (2x_2p);
# ACT ~1260 per col (1x always); GPSIMD tensor_scalar ~15.5ns/elem (NEVER
# use); GPSIMD memset ~free.  So: feature columns go DVE-first, ACT takes
# the A-row overflow, GPSIMD only memsets, and emission order staggers the
# writers of each tile (GP memsets, then ACT's Sign rows, then DVE's rows
# after filler work) to keep single-writer-per-tile windows.
# A-plane (pack-major, stationary): rows 1..A_DVE_ROWS on DVE, rows
# A_DVE_ROWS+1..15 on ACT (Sign, +-1 coded).  B-plane (cut-major, moving):
# all rows 1..15 on DVE with the lo-AND folded into each op (and,is_ge
# chain); ci8 conversion on ACT.
A_DVE_ROWS = 7
B_ACT_ROWS = 0
MGROUP = 16        # images per pass-B tile group

FP32 = mybir.dt.float32
BF16 = mybir.dt.bfloat16
I16 = mybir.dt.int16
U8 = mybir.dt.uint8
ALU = mybir.AluOpType
ACTF = mybir.ActivationFunctionType
MAGIC = 8388608.0  # 2^23


def _enc_pm():
    """Which rows are +-1 coded (ACT Sign)."""
    encA = np.zeros(16, dtype=bool)
    encB = np.zeros(16, dtype=bool)
    encA[A_DVE_ROWS + 1:] = True
    if B_ACT_ROWS:
        encB[16 - B_ACT_ROWS:] = True
    return encA, encB


# ---------------------------------------------------------------------------
# Pass A: histogram + ci8 cache
def build_hist_nc(nimg=IMGS_PER_CORE, cpi=CPI, group=GROUP):
    assert cpi % 8 == 0
    nc = bacc.Bacc("TRN2", target_bir_lowering=False, debug=False)
    x_d = nc.dram_tensor("x", [PARTS, nimg, cpi], FP32, kind="ExternalInput")
    w_d = nc.dram_tensor("w_raw", [nimg, PARTS, PARTS], FP32, kind="ExternalOutput")
    c8_d = nc.dram_tensor("ci8", [PARTS, nimg, cpi], U8, kind="ExternalOutput")

    with TileContext(nc) as tc:
        with (
            tc.tile_pool(name="const", bufs=1) as cpool,
            tc.tile_pool(name="xin", bufs=2) as xpool,
            tc.tile_pool(name="prep", bufs=2) as ppool,
            tc.tile_pool(name="thermo", bufs=2) as tpool,
            tc.tile_pool(name="psum", bufs=4, space="PSUM") as qpool,
        ):
            # per-partition biases for ACT Sign rows: sign(v - cut + 0.5)
            nACT = 15 - A_DVE_ROWS
            act_bias = cpool.tile([PARTS, max(nACT, 1)], FP32, tag="abias")
            ab_idx = {}
            j = 0
            for tau in range(A_DVE_ROWS + 1, 16):
                nc.vector.memset(act_bias[:, j:j + 1], 0.5 - 16 * tau)
                ab_idx[("A", tau)] = j
                j += 1

            n_groups = math.ceil(nimg / group)

            def emit_load_prep(g):
                """DMA x(g) in, then DVE floor prep.  CAUTION: DVE op0/op1
                chains are FUSED (no intermediate fp32 rounding — probe-
                verified), so the 255* multiply must be a LONE op to round
                fl(255x) before the magic add; the add-add chain itself is
                safe fused (s-0.5 exact, convert rounds nearest) or unfused
                (classic 2^23 magic)."""
                g0 = g * group
                g1 = min(g0 + group, nimg)
                gw = (g1 - g0) * cpi
                x_t = xpool.tile([PARTS, group * cpi], FP32, tag="x")
                # +8 pad: A-plane pack count must be EVEN for DVE 4x mode
                # (odd-count [1,8]-run writes drop to 2x — probe-measured)
                ci = ppool.tile([PARTS, group * cpi + 8], I16, tag="ci")
                nc.sync.dma_start(
                    out=x_t[:, :gw],
                    in_=bass.AP(x_d, g0 * cpi, [[nimg * cpi, PARTS], [1, gw]]),
                )
                nc.vector.tensor_scalar(
                    out=x_t[:, :gw], in0=x_t[:, :gw],
                    scalar1=255.0, scalar2=None, op0=ALU.mult,
                )
                nc.vector.tensor_scalar(
                    out=ci[:, :gw], in0=x_t[:, :gw],
                    scalar1=MAGIC - 0.5, scalar2=-MAGIC,
                    op0=ALU.add, op1=ALU.add,
                )
                lo = ppool.tile([PARTS, group * cpi + 8], I16, tag="lo")
                nc.vector.tensor_scalar(
                    out=lo[:, :gw], in0=ci[:, :gw],
                    scalar1=15, scalar2=None, op0=ALU.bitwise_and,
                )
                return ci, lo

            cur = emit_load_prep(0)
            for g in range(n_groups):
                g0 = g * group
                g1 = min(g0 + group, nimg)
                gw = (g1 - g0) * cpi
                gw8 = gw // 8
                gw8p = gw8 + (gw8 & 1)   # even pack count for DVE 4x
                ci, lo = cur

                ci8 = ppool.tile([PARTS, group * cpi], U8, tag="ci8")
                A_t = tpool.tile([PARTS, group * cpi // 8 + 1, 16, 8], BF16, tag="A")
                B_t = tpool.tile([PARTS, 16, group * cpi], BF16, tag="B")

                # ones rows first (GPSIMD, ~free, before other writers)
                nc.gpsimd.memset(A_t[:, :gw8p, 0, :], 1.0)
                nc.gpsimd.memset(B_t[:, 0, :gw], 1.0)

                ci_v = ci[:, :gw8p * 8].rearrange("p (a b) -> p a b", b=8)
                # ACT: Sign rows of the A plane + the ci8 u8 conversion.
                # Emitted before DVE's blocks so ACT is done with the A tile
                # by the time DVE's A rows issue (single-writer windows).
                for tau in range(A_DVE_ROWS + 1, 16):
                    j = ab_idx[("A", tau)]
                    nc.scalar.activation(
                        out=A_t[:, :gw8p, tau, :], in_=ci_v,
                        func=ACTF.Sign,
                        bias=act_bias[:, j:j + 1],
                        scale=1.0,
                    )
                nc.scalar.activation(
                    out=ci8[:, :gw], in_=ci[:, :gw],
                    func=ACTF.Copy, bias=0.0, scale=1.0,
                )
                nc.sync.dma_start(
                    out=bass.AP(c8_d, g0 * cpi, [[nimg * cpi, PARTS], [1, gw]]),
                    in_=ci8[:, :gw],
                )

                # DVE: B plane rows
                for sg in range(1, 16):
                    nc.vector.tensor_scalar(
                        out=B_t[:, sg, :gw], in0=lo[:, :gw],
                        scalar1=sg, scalar2=None, op0=ALU.is_ge,
                    )

                # filler between DVE's B and A blocks: next group's load+prep
                if g + 1 < n_groups:
                    cur = emit_load_prep(g + 1)

                # DVE: A plane rows (ACT has finished its A rows by now)
                for tau in range(1, A_DVE_ROWS + 1):
                    nc.vector.tensor_scalar(
                        out=A_t[:, :gw8p, tau, :], in0=ci_v,
                        scalar1=16 * tau, scalar2=None, op0=ALU.is_ge,
                    )

                # PE: per image, 49 packed [128,128] matmuls accumulate in PSUM
                packs_per_img = cpi // 8
                for i in range(g0, g1):
                    il = i - g0
                    psum_t = qpool.tile([PARTS, PARTS], FP32, tag="ps")
                    for k in range(packs_per_img):
                        p = il * packs_per_img + k
                        nc.tensor.matmul(
                            psum_t[:],
                            A_t[:, p, :, :].rearrange("p a b -> p (a b)"),
                            B_t[:, :, 8 * p:8 * p + 8],
                            start=(k == 0),
                            stop=(k == packs_per_img - 1),
                        )
                    w_sb = ppool.tile([PARTS, PARTS], FP32, tag="wsb")
                    nc.scalar.copy(w_sb[:], psum_t[:])
                    nc.sync.dma_start(out=w_d.ap()[i], in_=w_sb[:])
    nc.finalize()
    return nc


# ---------------------------------------------------------------------------
# Pass B: mask from cached ci8
def build_mask_nc(nimg=IMGS_PER_CORE, cpi=CPI, mgroup=MGROUP):
    nc = bacc.Bacc("TRN2", target_bir_lowering=False, debug=False)
    c8_d = nc.dram_tensor("ci8", [PARTS, nimg, cpi], U8, kind="ExternalInput")
    t_d = nc.dram_tensor("thp1", [PARTS, nimg], FP32, kind="ExternalInput")
    m_d = nc.dram_tensor("mask", [PARTS, nimg, cpi], U8, kind="ExternalOutput")

    with TileContext(nc) as tc:
        with (
            tc.tile_pool(name="cst", bufs=1) as cpool,
            tc.tile_pool(name="cin", bufs=3) as xpool,
            tc.tile_pool(name="mo", bufs=3) as mpool,
        ):
            th_all = cpool.tile([PARTS, nimg], FP32, tag="t")
            nc.sync.dma_start(out=th_all[:], in_=t_d.ap())
            for g0 in range(0, nimg, mgroup):
                g1 = min(g0 + mgroup, nimg)
                gl = g1 - g0
                c_t = xpool.tile([PARTS, mgroup * cpi], U8, tag="c")
                m_t = mpool.tile([PARTS, mgroup * cpi], U8, tag="m")
                nc.sync.dma_start(
                    out=c_t[:, :gl * cpi],
                    in_=bass.AP(
                        c8_d, g0 * cpi,
                        [[nimg * cpi, PARTS], [1, gl * cpi]],
                    ),
                )
                for i in range(g0, g1):
                    il = i - g0
                    nc.vector.tensor_scalar(
                        out=m_t[:, il * cpi:(il + 1) * cpi],
                        in0=c_t[:, il * cpi:(il + 1) * cpi],
                        scalar1=th_all[:, i:i + 1],
                        scalar2=None, op0=ALU.is_ge,
                    )
                nc.sync.dma_start(
                    out=bass.AP(
                        m_d, g0 * cpi,
                        [[nimg * cpi, PARTS], [1, gl * cpi]],
                    ),
                    in_=m_t[:, :gl * cpi],
                )
    nc.finalize()
    return nc


# ---------------------------------------------------------------------------
# Host: decode W, exact-float32 Otsu
def decode_hist(w_raw, nimg=IMGS_PER_CORE, npix=PIX):
    """w_raw [nimg, 128, 128] fp32 -> hist [nimg, 256] int64 (exact).

    Psum row 8*tau+c', col 8*sigma+c'': sum the c'==c'' diagonal blocks."""
    encA, encB = _enc_pm()
    P128 = np.round(np.asarray(w_raw, np.float64)).astype(np.int64)
    P128 = P128.reshape(nimg, 16, 8, 16, 8)  # [img, tau, c', sigma, c'']
    R = np.einsum("itcsc->its", P128)        # [img, tau, sigma]
    P = npix
    sumB = np.where(encB[None, :], (R[:, 0, :] + P) // 2, R[:, 0, :])
    sumA = np.where(encA[None, :], (R[:, :, 0] + P) // 2, R[:, :, 0])
    eA = encA[None, :, None]
    eB = encB[None, None, :]
    sA = sumA[:, :, None]
    sB = sumB[:, None, :]
    W = np.where(
        ~eA & ~eB, R,
        np.where(
            eA & ~eB, (R + sB) // 2,
            np.where(~eA & eB, (R + sA) // 2, (R + 2 * sA + 2 * sB - P) // 4),
        ),
    )
    chk = np.where(
        ~eA & ~eB, 0,
        np.where(eA & ~eB, (R + sB) % 2,
                 np.where(~eA & eB, (R + sA) % 2, (R + 2 * sA + 2 * sB - P) % 4)),
    )
    assert not chk.any(), "non-integer decode: device histogram corrupted"
    Wp = np.zeros((nimg, 17, 17), np.int64)
    Wp[:, :16, :16] = W
    hist = (Wp[:, :16, :16] - Wp[:, 1:, :16] - Wp[:, :16, 1:] + Wp[:, 1:, 1:])
    hist = hist.reshape(nimg, 256)
    assert (hist >= 0).all() and (hist.sum(1) == P).all(), "bad histogram"
    return hist


def otsu_f32(hist):
    """Mirror the jax float32 reference exactly. hist [n,256] int64 -> th, bad."""
    f = hist.astype(np.float32)
    centers = np.arange(NBINS, dtype=np.float32)
    w1 = np.cumsum(f, axis=1, dtype=np.float32)
    total = w1[:, -1:]
    s1 = np.cumsum(f * centers, axis=1, dtype=np.float32)
    stot = s1[:, -1:]
    w2 = total - w1
    with np.errstate(divide="ignore", invalid="ignore"):
        m1 = s1 / w1
        m2 = (stot - s1) / w2
        d = m1 - m2
        var12 = (w1 * w2) * (d * d)
    nz = hist > 0
    t = np.arange(NBINS)
    vmin = np.argmax(nz, axis=1)
    vmax = NBINS - 1 - np.argmax(nz[:, ::-1], axis=1)
    valid = (t[None, :] >= vmin[:, None]) & (t[None, :] < vmax[:, None])
    var12 = np.where(valid, var12, np.float32(-1.0))
    th = np.argmax(var12, axis=1)
    th = np.where(th == 0, 1, th)
    th = np.where(th == 255, 254, th)
    bad = vmin == vmax
    return th, bad


# ---------------------------------------------------------------------------
_NC_CACHE = {}


def _get_ncs():
    if "hist" not in _NC_CACHE:
        _NC_CACHE["hist"] = build_hist_nc()
        _NC_CACHE["mask"] = build_mask_nc()
    return _NC_CACHE["hist"], _NC_CACHE["mask"]


def kernel(x: np.ndarray, _profile: dict | None = None) -> np.ndarray:
    x = np.ascontiguousarray(np.asarray(x, dtype=np.float32))
    assert x.shape == (B, N, H, W_IMG)
    nc_hist, nc_mask = _get_ncs()

    bpc = B // N_CORES
    shards = [
        np.ascontiguousarray(
            x[k * bpc:(k + 1) * bpc]
            .reshape(IMGS_PER_CORE, PARTS, CPI)
            .transpose(1, 0, 2)
        )
        for k in range(N_CORES)
    ]
    core_ids = list(range(N_CORES))

    kwargs_a = dict(_profile.get("a", {})) if _profile else {}
    res_a = run_bass_kernel_spmd(
        nc_hist, [{"x": s} for s in shards], core_ids=core_ids, **kwargs_a
    )
    if _profile is not None:
        _profile["res_a"] = res_a

    thp1s = []
    for k in range(N_CORES):
        hist = decode_hist(res_a.results[k]["w_raw"])
        th, bad = otsu_f32(hist)
        thp1 = np.where(bad, np.float32(256.0), (th + 1).astype(np.float32))
        thp1s.append(
            np.ascontiguousarray(
                np.broadcast_to(thp1[None, :], (PARTS, IMGS_PER_CORE))
            ).astype(np.float32)
        )

    kwargs_b = dict(_profile.get("b", {})) if _profile else {}
    res_b = run_bass_kernel_spmd(
        nc_mask,
        [{"ci8": np.asarray(res_a.results[k]["ci8"]), "thp1": thp1s[k]}
         for k in range(N_CORES)],
        core_ids=core_ids,
        **kwargs_b,
    )
    if _profile is not None:
        _profile["res_b"] = res_b

    out = np.empty((B, N, H, W_IMG), np.int32)
    for k in range(N_CORES):
        m = np.asarray(res_b.results[k]["mask"])  # [128, 64, 392] u8
        out[k * bpc:(k + 1) * bpc] = (
            m.astype(np.int32).transpose(1, 0, 2).reshape(bpc, N, H, W_IMG)
        )
    return out


# revision 14
# speedup vs baseline: 1.1418x; 1.0024x over previous
"""Otsu-threshold binary region proposal kernel for Trainium2 (8 NeuronCores).

Algorithm (per image of 224*224 pixels, 512 images total, data-parallel over
8 cores / 64 images per core):

  reference:  cam = floor(x*255); per-image 256-bin histogram; Otsu threshold
              via argmax of inter-class variance restricted to [vmin, vmax);
              roi = (cam > th), 0 for degenerate images.

Device pass A (histogram + cam cache):
  Thermometer decomposition: with hi = cam >> 4, lo = cam & 15,
      R[tau, sigma] = sum_p colA_tau(p) * colB_sigma(p)
  colA_tau ~ [cam >= 16*tau] (pack-major, stationary), colB_sigma ~ [lo >=
  sigma] (cut-major, moving); 16x16 pair counts accumulate on the PE as one
  [128,128]x[128,128] matmul per 8-chunk pack (block-diagonal; host sums the
  c'==c'' blocks).  Engine split is tuned so DVE / ACT / GPSIMD all sit near
  the PE's ~210us roofline: DVE takes the fused floor prep (255x + magic in
  2 ops), the contiguous B rows (4x mode) and a few strided A rows (2x);
  ACT takes the remaining A rows (Sign, +-1 coded); GPSIMD takes the memset
  ones-rows and a slice of contiguous B rows.  Pass A also emits cam as
  uint8 (ci8) to DRAM so pass B never re-reads x.

Host (exact float32, mirrors jax reference op-for-op):
  decode W -> 256-bin histogram -> cumsums -> inter-class variance -> argmax
  -> th.  mask = (cam > th) == (ci8 >= th+1), exact in integers: no float
  cut table needed.  Degenerate images get th+1 = 256 (never fires).

Device pass B (mask): mask = (ci8 >= thp1) as uint8; reads 3.2MB instead of
the 12.8MB of x, writes 3.2MB.

floor() trick (no floor ALU op): t = fl(255x + (2^23 - 0.5)); ci = t - 2^23
= floor(fl(255x)) exactly (fp32 ulp 1.0 at 2^23 forces round-to-integer;
the -0.5 biases the tie so exact integers round down correctly... for
s = fl(255x) integer the only case is s=0 -> t = 2^23 - 0.5 rounds to 2^23
(ties-to-even) -> ci = 0 correct).
"""

import math
import os
import sys

import numpy as np

sys.path.insert(0, "/opt/trn_rl_repo")

import concourse.bacc as bacc
import concourse.bass as bass  # noqa: F401
import concourse.mybir as mybir
from concourse.bass_utils import run_bass_kernel_spmd
from concourse.tile import TileContext

# ---------------------------------------------------------------------------
# Problem geometry (hardcoded per spec)
B, N, H, W_IMG = 64, 8, 224, 224
PIX = H * W_IMG              # 50176
PARTS = 128
CPI = PIX // PARTS           # 392 chunks (columns) per image
N_CORES = 8
IMGS_PER_CORE = (B // N_CORES) * N      # 64
NBINS = 256

# Tunables -----------------------------------------------------------------
GROUP = 3          # images per thermo group
# Probe-measured (ns, FD1176-equivalent): DVE is_ge bf16-out ~457 at 4x
# (contiguous OR [1,8]-run strided) but degrades to ~755 (2x) when ANOTHER
# engine is concurrently writing the same SBUF tile (port contention — same
# mechanism the GPSIMD note below describes).  DVE u8/fp32 ops ~755 (2x_2p);
# ACT ~1260 per col (1x always); GPSIMD tensor_scalar ~15.5ns/elem (NEVER
# use); GPSIMD memset ~free.  So: feature columns go DVE-first, ACT takes
# the A-row overflow, GPSIMD only memsets, and emission order staggers the
# writers of each tile (GP memsets, then ACT's Sign rows, then DVE's rows
# after filler work) to keep single-writer-per-tile windows.
# A-plane (pack-major, stationary): rows 1..A_DVE_ROWS on DVE, rows
# A_DVE_ROWS+1..15 on ACT (Sign, +-1 coded).  B-plane (cut-major, moving):
# all rows 1..15 on DVE with the lo-AND folded into each op (and,is_ge
# chain); ci8 conversion on ACT.
A_DVE_ROWS = 7
B_ACT_ROWS = 0
MGROUP = 16        # images per pass-B tile group

FP32 = mybir.dt.float32
BF16 = mybir.dt.bfloat16
I16 = mybir.dt.int16
U8 = mybir.dt.uint8
ALU = mybir.AluOpType
ACTF = mybir.ActivationFunctionType
MAGIC = 8388608.0  # 2^23


def _enc_pm():
    """Which rows are +-1 coded (ACT Sign)."""
    encA = np.zeros(16, dtype=bool)
    encB = np.zeros(16, dtype=bool)
    encA[A_DVE_ROWS + 1:] = True
    if B_ACT_ROWS:
        encB[16 - B_ACT_ROWS:] = True
    return encA, encB


# ---------------------------------------------------------------------------
# Pass A: histogram + ci8 cache
def build_hist_nc(nimg=IMGS_PER_CORE, cpi=CPI, group=GROUP):
    assert cpi % 8 == 0
    nc = bacc.Bacc("TRN2", target_bir_lowering=False, debug=False)
    x_d = nc.dram_tensor("x", [PARTS, nimg, cpi], FP32, kind="ExternalInput")
    w_d = nc.dram_tensor("w_raw", [nimg, PARTS, PARTS], FP32, kind="ExternalOutput")
    c8_d = nc.dram_tensor("ci8", [PARTS, nimg, cpi], U8, kind="ExternalOutput")

    with TileContext(nc) as tc:
        with (
            tc.tile_pool(name="const", bufs=1) as cpool,
            tc.tile_pool(name="xin", bufs=3) as xpool,
            tc.tile_pool(name="prep", bufs=3) as ppool,
            tc.tile_pool(name="thermo", bufs=2) as tpool,
            tc.tile_pool(name="psum", bufs=8, space="PSUM") as qpool,
        ):
            # per-partition biases for ACT Sign rows: sign(v - cut + 0.5)
            nACT = 15 - A_DVE_ROWS
            act_bias = cpool.tile([PARTS, max(nACT, 1)], FP32, tag="abias")
            ab_idx = {}
            j = 0
            for tau in range(A_DVE_ROWS + 1, 16):
                nc.vector.memset(act_bias[:, j:j + 1], 0.5 - 16 * tau)
                ab_idx[("A", tau)] = j
                j += 1

            n_groups = math.ceil(nimg / group)

            def emit_load(g):
                g0 = g * group
                g1 = min(g0 + group, nimg)
                gw = (g1 - g0) * cpi
                x_t = xpool.tile([PARTS, group * cpi], FP32, tag="x")
                nc.sync.dma_start(
                    out=x_t[:, :gw],
                    in_=bass.AP(x_d, g0 * cpi, [[nimg * cpi, PARTS], [1, gw]]),
                )
                return x_t

            def emit_prep(g, x_t):
                """DVE floor prep.  CAUTION: DVE op0/op1 chains are FUSED
                (no intermediate fp32 rounding — probe-verified), so the
                255* multiply must be a LONE op to round fl(255x) before
                the magic add; the add-add chain itself is safe fused
                (s-0.5 exact, convert rounds nearest) or unfused (classic
                2^23 magic)."""
                g0 = g * group
                g1 = min(g0 + group, nimg)
                gw = (g1 - g0) * cpi
                # +8 pad: A-plane pack count must be EVEN for DVE 4x mode
                # (odd-count [1,8]-run writes drop to 2x — probe-measured)
                ci = ppool.tile([PARTS, group * cpi + 8], I16, tag="ci")
                nc.vector.tensor_scalar(
                    out=x_t[:, :gw], in0=x_t[:, :gw],
                    scalar1=255.0, scalar2=None, op0=ALU.mult,
                )
                nc.vector.tensor_scalar(
                    out=ci[:, :gw], in0=x_t[:, :gw],
                    scalar1=MAGIC - 0.5, scalar2=-MAGIC,
                    op0=ALU.add, op1=ALU.add,
                )
                lo = ppool.tile([PARTS, group * cpi + 8], I16, tag="lo")
                nc.vector.tensor_scalar(
                    out=lo[:, :gw], in0=ci[:, :gw],
                    scalar1=15, scalar2=None, op0=ALU.bitwise_and,
                )
                return ci, lo

            x_cur = emit_load(0)
            cur = emit_prep(0, x_cur)
            x_nxt = emit_load(1) if n_groups > 1 else None
            for g in range(n_groups):
                g0 = g * group
                g1 = min(g0 + group, nimg)
                gw = (g1 - g0) * cpi
                gw8 = gw // 8
                gw8p = gw8 + (gw8 & 1)   # even pack count for DVE 4x
                ci, lo = cur

                if g + 2 < n_groups:
                    x_nxt2 = emit_load(g + 2)
                else:
                    x_nxt2 = None

                ci8 = ppool.tile([PARTS, group * cpi], U8, tag="ci8")
                A_t = tpool.tile([PARTS, group * cpi // 8 + 1, 16, 8], BF16, tag="A")
                B_t = tpool.tile([PARTS, 16, group * cpi], BF16, tag="B")

                # ones rows first (GPSIMD, ~free, before other writers)
                nc.gpsimd.memset(A_t[:, :gw8p, 0, :], 1.0)
                nc.gpsimd.memset(B_t[:, 0, :gw], 1.0)

                ci_v = ci[:, :gw8p * 8].rearrange("p (a b) -> p a b", b=8)
                # ACT: Sign rows of the A plane + the ci8 u8 conversion.
                # Emitted before DVE's blocks so ACT is done with the A tile
                # by the time DVE's A rows issue (single-writer windows).
                for tau in range(A_DVE_ROWS + 1, 16):
                    j = ab_idx[("A", tau)]
                    nc.scalar.activation(
                        out=A_t[:, :gw8p, tau, :], in_=ci_v,
                        func=ACTF.Sign,
                        bias=act_bias[:, j:j + 1],
                        scale=1.0,
                    )
                nc.scalar.activation(
                    out=ci8[:, :gw], in_=ci[:, :gw],
                    func=ACTF.Copy, bias=0.0, scale=1.0,
                )
                nc.sync.dma_start(
                    out=bass.AP(c8_d, g0 * cpi, [[nimg * cpi, PARTS], [1, gw]]),
                    in_=ci8[:, :gw],
                )

                # DVE: B plane rows
                for sg in range(1, 16):
                    nc.vector.tensor_scalar(
                        out=B_t[:, sg, :gw], in0=lo[:, :gw],
                        scalar1=sg, scalar2=None, op0=ALU.is_ge,
                    )

                # filler between DVE's B and A blocks: next group's prep
                if g + 1 < n_groups:
                    cur = emit_prep(g + 1, x_nxt)
                x_nxt = x_nxt2

                # DVE: A plane rows (ACT has finished its A rows by now)
                for tau in range(1, A_DVE_ROWS + 1):
                    nc.vector.tensor_scalar(
                        out=A_t[:, :gw8p, tau, :], in0=ci_v,
                        scalar1=16 * tau, scalar2=None, op0=ALU.is_ge,
                    )

                # PE: per image, 49 packed [128,128] matmuls accumulate in PSUM
                packs_per_img = cpi // 8
                for i in range(g0, g1):
                    il = i - g0
                    psum_t = qpool.tile([PARTS, PARTS], FP32, tag="ps")
                    for k in range(packs_per_img):
                        p = il * packs_per_img + k
                        nc.tensor.matmul(
                            psum_t[:],
                            A_t[:, p, :, :].rearrange("p a b -> p (a b)"),
                            B_t[:, :, 8 * p:8 * p + 8],
                            start=(k == 0),
                            stop=(k == packs_per_img - 1),
                        )
                    w_sb = ppool.tile([PARTS, PARTS], FP32, tag="wsb")
                    nc.scalar.copy(w_sb[:], psum_t[:])
                    nc.sync.dma_start(out=w_d.ap()[i], in_=w_sb[:])
    nc.finalize()
    return nc


# ---------------------------------------------------------------------------
# Pass B: mask from cached ci8
def build_mask_nc(nimg=IMGS_PER_CORE, cpi=CPI, mgroup=MGROUP):
    nc = bacc.Bacc("TRN2", target_bir_lowering=False, debug=False)
    c8_d = nc.dram_tensor("ci8", [PARTS, nimg, cpi], U8, kind="ExternalInput")
    t_d = nc.dram_tensor("thp1", [PARTS, nimg], FP32, kind="ExternalInput")
    m_d = nc.dram_tensor("mask", [PARTS, nimg, cpi], U8, kind="ExternalOutput")

    with TileContext(nc) as tc:
        with (
            tc.tile_pool(name="cst", bufs=1) as cpool,
            tc.tile_pool(name="cin", bufs=3) as xpool,
            tc.tile_pool(name="mo", bufs=3) as mpool,
        ):
            th_all = cpool.tile([PARTS, nimg], FP32, tag="t")
            nc.sync.dma_start(out=th_all[:], in_=t_d.ap())
            for g0 in range(0, nimg, mgroup):
                g1 = min(g0 + mgroup, nimg)
                gl = g1 - g0
                c_t = xpool.tile([PARTS, mgroup * cpi], U8, tag="c")
                m_t = mpool.tile([PARTS, mgroup * cpi], U8, tag="m")
                nc.sync.dma_start(
                    out=c_t[:, :gl * cpi],
                    in_=bass.AP(
                        c8_d, g0 * cpi,
                        [[nimg * cpi, PARTS], [1, gl * cpi]],
                    ),
                )
                for i in range(g0, g1):
                    il = i - g0
                    nc.vector.tensor_scalar(
                        out=m_t[:, il * cpi:(il + 1) * cpi],
                        in0=c_t[:, il * cpi:(il + 1) * cpi],
                        scalar1=th_all[:, i:i + 1],
                        scalar2=None, op0=ALU.is_ge,
                    )
                nc.sync.dma_start(
                    out=bass.AP(
                        m_d, g0 * cpi,
                        [[nimg * cpi, PARTS], [1, gl * cpi]],
                    ),
                    in_=m_t[:, :gl * cpi],
                )
    nc.finalize()
    return nc


# ---------------------------------------------------------------------------
# Host: decode W, exact-float32 Otsu
def decode_hist(w_raw, nimg=IMGS_PER_CORE, npix=PIX):
    """w_raw [nimg, 128, 128] fp32 -> hist [nimg, 256] int64 (exact).

    Psum row 8*tau+c', col 8*sigma+c'': sum the c'==c'' diagonal blocks."""
    encA, encB = _enc_pm()
    P128 = np.round(np.asarray(w_raw, np.float64)).astype(np.int64)
    P128 = P128.reshape(nimg, 16, 8, 16, 8)  # [img, tau, c', sigma, c'']
    R = np.einsum("itcsc->its", P128)        # [img, tau, sigma]
    P = npix
    sumB = np.where(encB[None, :], (R[:, 0, :] + P) // 2, R[:, 0, :])
    sumA = np.where(encA[None, :], (R[:, :, 0] + P) // 2, R[:, :, 0])
    eA = encA[None, :, None]
    eB = encB[None, None, :]
    sA = sumA[:, :, None]
    sB = sumB[:, None, :]
    W = np.where(
        ~eA & ~eB, R,
        np.where(
            eA & ~eB, (R + sB) // 2,
            np.where(~eA & eB, (R + sA) // 2, (R + 2 * sA + 2 * sB - P) // 4),
        ),
    )
    chk = np.where(
        ~eA & ~eB, 0,
        np.where(eA & ~eB, (R + sB) % 2,
                 np.where(~eA & eB, (R + sA) % 2, (R + 2 * sA + 2 * sB - P) % 4)),
    )
    assert not chk.any(), "non-integer decode: device histogram corrupted"
    Wp = np.zeros((nimg, 17, 17), np.int64)
    Wp[:, :16, :16] = W
    hist = (Wp[:, :16, :16] - Wp[:, 1:, :16] - Wp[:, :16, 1:] + Wp[:, 1:, 1:])
    hist = hist.reshape(nimg, 256)
    assert (hist >= 0).all() and (hist.sum(1) == P).all(), "bad histogram"
    return hist


def otsu_f32(hist):
    """Mirror the jax float32 reference exactly. hist [n,256] int64 -> th, bad."""
    f = hist.astype(np.float32)
    centers = np.arange(NBINS, dtype=np.float32)
    w1 = np.cumsum(f, axis=1, dtype=np.float32)
    total = w1[:, -1:]
    s1 = np.cumsum(f * centers, axis=1, dtype=np.float32)
    stot = s1[:, -1:]
    w2 = total - w1
    with np.errstate(divide="ignore", invalid="ignore"):
        m1 = s1 / w1
        m2 = (stot - s1) / w2
        d = m1 - m2
        var12 = (w1 * w2) * (d * d)
    nz = hist > 0
    t = np.arange(NBINS)
    vmin = np.argmax(nz, axis=1)
    vmax = NBINS - 1 - np.argmax(nz[:, ::-1], axis=1)
    valid = (t[None, :] >= vmin[:, None]) & (t[None, :] < vmax[:, None])
    var12 = np.where(valid, var12, np.float32(-1.0))
    th = np.argmax(var12, axis=1)
    th = np.where(th == 0, 1, th)
    th = np.where(th == 255, 254, th)
    bad = vmin == vmax
    return th, bad


# ---------------------------------------------------------------------------
_NC_CACHE = {}


def _get_ncs():
    if "hist" not in _NC_CACHE:
        _NC_CACHE["hist"] = build_hist_nc()
        _NC_CACHE["mask"] = build_mask_nc()
    return _NC_CACHE["hist"], _NC_CACHE["mask"]


def kernel(x: np.ndarray, _profile: dict | None = None) -> np.ndarray:
    x = np.ascontiguousarray(np.asarray(x, dtype=np.float32))
    assert x.shape == (B, N, H, W_IMG)
    nc_hist, nc_mask = _get_ncs()

    bpc = B // N_CORES
    shards = [
        np.ascontiguousarray(
            x[k * bpc:(k + 1) * bpc]
            .reshape(IMGS_PER_CORE, PARTS, CPI)
            .transpose(1, 0, 2)
        )
        for k in range(N_CORES)
    ]
    core_ids = list(range(N_CORES))

    kwargs_a = dict(_profile.get("a", {})) if _profile else {}
    res_a = run_bass_kernel_spmd(
        nc_hist, [{"x": s} for s in shards], core_ids=core_ids, **kwargs_a
    )
    if _profile is not None:
        _profile["res_a"] = res_a

    thp1s = []
    for k in range(N_CORES):
        hist = decode_hist(res_a.results[k]["w_raw"])
        th, bad = otsu_f32(hist)
        thp1 = np.where(bad, np.float32(256.0), (th + 1).astype(np.float32))
        thp1s.append(
            np.ascontiguousarray(
                np.broadcast_to(thp1[None, :], (PARTS, IMGS_PER_CORE))
            ).astype(np.float32)
        )

    kwargs_b = dict(_profile.get("b", {})) if _profile else {}
    res_b = run_bass_kernel_spmd(
        nc_mask,
        [{"ci8": np.asarray(res_a.results[k]["ci8"]), "thp1": thp1s[k]}
         for k in range(N_CORES)],
        core_ids=core_ids,
        **kwargs_b,
    )
    if _profile is not None:
        _profile["res_b"] = res_b

    out = np.empty((B, N, H, W_IMG), np.int32)
    for k in range(N_CORES):
        m = np.asarray(res_b.results[k]["mask"])  # [128, 64, 392] u8
        out[k * bpc:(k + 1) * bpc] = (
            m.astype(np.int32).transpose(1, 0, 2).reshape(bpc, N, H, W_IMG)
        )
    return out
